# revision 11
# baseline (speedup 1.0000x reference)
"""Trainium2 Bass kernel for the pairwise+triplewise cycle-consistency loss.

Strategy (8 NeuronCores, tensor-parallel over rows of each [N,N] block):
  - All six cycle-term matrices have the form  A = U @ nf_j^T  with
    U = nf_i (pairs) or U = nf_i @ G_k (triples), G_k = nf_k^T nf_k [D,D],
    collapsing the [N,N]@[N,N] triple products into [D,D] Gram matmuls.
  - Each core owns a 512-row block R_c and computes A[R_c,:] ONCE (f32r
    matmuls, bf16 result). S12_hat rows come from a local row-softmax.
    S21_hat (the column softmax) is derived from PE-transposed A tiles
    plus two tiny [128,32] AllReduces (col-max, col-sum) — the second
    [RPC,D]@[D,N] matmul set of the baseline is gone.
  - Both M-product operands are quantized to fp8e4 (values in [0,1]);
    S21_hat columns are AllGathered as a [N, RPC] fp8 payload and the
    M^T column-tiles are computed with DoubleRow fp8 matmuls (2x rate),
    accumulating rowmax/colmax/diag stats on the fly. Host assembles the
    scalar loss.
"""
import sys
sys.path.insert(0, "/opt/trn_rl_repo")

import math
import numpy as np

import concourse.bass as bass
import concourse.mybir as mybir
import concourse.tile as tile
from concourse import bacc
from concourse.bass_utils import run_bass_kernel_spmd
from concourse.masks import make_identity

F32 = mybir.dt.float32
F32R = mybir.dt.float32r
BF16 = mybir.dt.bfloat16
FP8 = mybir.dt.float8e4
AX = mybir.AxisListType
OP = mybir.AluOpType
ACT = mybir.ActivationFunctionType
DR = mybir.MatmulPerfMode.DoubleRow

NTOK = 4096          # rows per view
D = 1024             # feature dim
NC = 8               # cores
RPC = NTOK // NC     # rows per core (512)
P = 128
NRT = RPC // P       # rowtiles per core (4)
NS = 8               # 512-col strips of A
DKB = D // P         # d-blocks (8)
NKB = NTOK // P      # k-tiles (32)
SCALE = math.log(NTOK) / 0.1
MARGIN = 0.5

# term table: (is_tri, gram_idx, lhsA, rhsA); lhs indexes x_i, rhs indexes f_i.
# For tri terms lhs is G[gram_idx] @ x_i.
TERMS = [
    (False, None, 0, 1),   # S01
    (False, None, 0, 2),   # S02
    (False, None, 1, 2),   # S12
    (True, 2, 0, 1),       # S02 @ S21 = nf0 G2 nf1^T
    (True, 1, 0, 2),       # S01 @ S12 = nf0 G1 nf2^T
    (True, 0, 1, 2),       # S10 @ S02 = nf1 G0 nf2^T
]

OUT_W = RPC + 32 + NRT   # racc 512 | colmax32 32 | diag 4


def build_program():
    nc = bacc.Bacc("TRN2", target_bir_lowering=False, debug=False, num_devices=NC)

    xs = [nc.dram_tensor(f"x{i}", [D, RPC], F32R, kind="ExternalInput") for i in range(3)]
    ws = [nc.dram_tensor(f"w{i}", [RPC, D], F32R, kind="ExternalInput") for i in range(3)]
    fs = [nc.dram_tensor(f"f{i}", [D, NTOK], F32R, kind="ExternalInput") for i in range(3)]
    wsel_in = nc.dram_tensor("wsel", [P, P], F32, kind="ExternalInput")
    out = nc.dram_tensor("out", [6, P, OUT_W], F32, kind="ExternalOutput")

    with tile.TileContext(nc) as tc:
        with (
            tc.tile_pool(name="cst", bufs=1) as cst,
            tc.tile_pool(name="lhs", bufs=2) as lhsp,
            tc.tile_pool(name="rhs", bufs=2) as rhsp,
            tc.tile_pool(name="abf", bufs=4) as abfp,
            tc.tile_pool(name="at", bufs=1) as atp,
            tc.tile_pool(name="pt", bufs=3) as ptp,
            tc.tile_pool(name="stg", bufs=2) as stgp,
            tc.tile_pool(name="qsb", bufs=2) as qsbp,
            tc.tile_pool(name="st", bufs=2) as stp,
            tc.tile_pool(name="sm", bufs=4) as smp,
            tc.tile_pool(name="psA", bufs=2, space="PSUM") as psA,
            tc.tile_pool(name="psT", bufs=2, space="PSUM") as psT,
            tc.tile_pool(name="psM", bufs=4, space="PSUM") as psM,
            tc.tile_pool(name="dram", bufs=1, space="DRAM") as dram,
            tc.tile_pool(name="dram2", bufs=3, space="DRAM") as dram2,
            tc.tile_pool(name="dram3", bufs=2, space="DRAM") as dram3,
        ):
            # constants
            identb = cst.tile([P, P], BF16)
            make_identity(nc, identb)
            wsel = cst.tile([P, P], F32)
            nc.sync.dma_start(wsel[:], wsel_in[:])
            nwsel = cst.tile([P, P], F32)
            nc.vector.tensor_scalar_mul(nwsel[:], wsel[:], -1.0)
            # imask4[p, 128b+p] = 1 for b in 0..3 (diag candidate positions)
            identf = cst.tile([P, P], F32)
            make_identity(nc, identf)
            imask4 = cst.tile([P, NRT, P], F32)
            for b in range(NRT):
                nc.vector.tensor_copy(imask4[:, b, :], identf[:])

            # ---------------- Gram phase ----------------
            gins = [dram.tile([D, D], F32, tag=f"gin{k}", name=f"gin{k}")
                    for k in range(3)]
            gouts = [dram.tile([D, D], F32, tag=f"gout{k}", addr_space="Shared",
                               name=f"gout{k}") for k in range(3)]
            for k in range(3):
                w_sb = lhsp.tile([P, NRT, D], F32R, tag="lhs", name=f"w_sb{k}")
                nc.sync.dma_start(w_sb[:], ws[k].rearrange("(o p) d -> p o d", p=P))
                for d1 in range(DKB):
                    for d2 in range(2):
                        ps = psA.tile([P, 512], F32, tag="psA", name=f"gps{k}_{d1}_{d2}")
                        for nt in range(NRT):
                            nc.tensor.matmul(
                                ps[:], w_sb[:, nt, d1 * P:(d1 + 1) * P],
                                w_sb[:, nt, d2 * 512:(d2 + 1) * 512],
                                start=(nt == 0), stop=(nt == NRT - 1))
                        gtmp = stp.tile([P, 512], F32, tag="gtmp", name=f"gt{k}_{d1}_{d2}")
                        nc.scalar.copy(gtmp[:], ps[:])
                        nc.sync.dma_start(
                            gins[k][d1 * P:(d1 + 1) * P,
                                    d2 * 512:(d2 + 1) * 512], gtmp[:])

            def kick_gram_ar(k):
                nc.gpsimd.collective_compute(
                    "AllReduce", OP.add, replica_groups=[list(range(NC))],
                    ins=[gins[k][:]], outs=[gouts[k][:]])

            # ---------------- helpers ----------------
            def load_x(i, nm):
                t = lhsp.tile([P, DKB, RPC], F32R, tag="lhs", name=f"x_{nm}")
                nc.sync.dma_start(t[:], xs[i].rearrange("(o p) r -> p o r", p=P))
                return t

            def compute_ut(gk, i, nm):
                """U^T[:, R_c] = G_k @ x_i  -> [128, DKB, RPC] f32r tile."""
                x_sb = load_x(i, f"utx_{nm}")
                ut = lhsp.tile([P, DKB, RPC], F32R, tag="lhs", name=f"ut_{nm}")
                for grp in range(2):
                    pss = [psM.tile([P, 512], F32, tag="psM", name=f"utps_{nm}_{grp}_{d4}")
                           for d4 in range(4)]
                    for half in range(2):
                        gh = rhsp.tile([P, 4, D], F32R, tag="rhs", name=f"gh_{nm}_{grp}_{half}")
                        nc.sync.dma_start(
                            gh[:], gouts[gk][half * 512:(half + 1) * 512]
                            .rearrange("(o p) d -> p o d", p=P).bitcast(F32R))
                        for d4 in range(4):
                            dp = 4 * grp + d4
                            for db in range(4):
                                nc.tensor.matmul(
                                    pss[d4][:], gh[:, db, dp * P:(dp + 1) * P],
                                    x_sb[:, 4 * half + db, :],
                                    start=(half == 0 and db == 0),
                                    stop=(half == 1 and db == 3))
                    for d4 in range(4):
                        nc.scalar.copy(ut[:, 4 * grp + d4, :], pss[d4][:])
                return ut

            def side_chunk(t, lhs_t, fj):
                """A[R_c, :] raw logits (pre-scale) as 4 bf16 quarter tiles."""
                chunk = [abfp.tile([P, NTOK], BF16, tag="abf", name=f"ch_{t}_{rt}")
                         for rt in range(NRT)]
                for s in range(NS):
                    rsb = rhsp.tile([P, DKB, 512], F32R, tag="rhs", name=f"rs_{t}_{s}")
                    nc.sync.dma_start(
                        rsb[:], fs[fj][:, s * 512:(s + 1) * 512]
                        .rearrange("(o p) n -> p o n", p=P))
                    for rt in range(NRT):
                        ps = psA.tile([P, 512], F32, tag="psA", name=f"aps_{t}_{s}_{rt}")
                        for kb in range(DKB):
                            nc.tensor.matmul(
                                ps[:], lhs_t[:, kb, rt * P:(rt + 1) * P],
                                rsb[:, kb, :], start=(kb == 0), stop=(kb == DKB - 1))
                        nc.scalar.copy(chunk[rt][:, s * 512:(s + 1) * 512], ps[:])
                return chunk

            def transpose_quarters(t, chunk, dst, nm):
                """PE-transpose chunk[rt] (4x [P, NTOK] bf16) into dst [P, NKB, RPC]."""
                for rt in range(NRT):
                    for g in range(NKB // 4):
                        tp = psT.tile([P, 512], BF16, tag="psT", name=f"tp{nm}_{t}_{rt}_{g}")
                        for q in range(4):
                            kb = 4 * g + q
                            nc.tensor.transpose(
                                tp[:, q * P:(q + 1) * P],
                                chunk[rt][:, kb * P:(kb + 1) * P], identb[:])
                        nc.vector.tensor_copy(
                            dst[:, 4 * g:4 * g + 4, rt * P:(rt + 1) * P],
                            tp.rearrange("p (o q) -> p o q", q=P))

            def col_side(t, chunk):
                """S21_hat columns [NTOK, R_c] -> fp8 allgather payload."""
                ag_in = dram2.tile([NTOK, RPC], FP8, tag="agin", name=f"agin{t}")
                ag_out = dram2.tile([NC * NTOK, RPC], FP8, tag="agout",
                                    addr_space="Shared", name=f"agout{t}")
                at_sb = atp.tile([P, NKB, RPC], BF16, tag="at", name=f"at{t}")
                transpose_quarters(t, chunk, at_sb, "c")
                # local col-max -> AllReduce max
                cm_loc = smp.tile([P, NKB], F32, tag="sm", name=f"cml{t}")
                nc.vector.reduce_max(cm_loc[:], at_sb[:], axis=AX.X)
                cm_in = dram3.tile([P, NKB], F32, tag="cmin", name=f"cmin{t}")
                cm_out = dram3.tile([P, NKB], F32, tag="cmout", addr_space="Shared",
                                    name=f"cmout{t}")
                nc.sync.dma_start(cm_in[:], cm_loc[:])
                nc.gpsimd.collective_compute(
                    "AllReduce", OP.max, replica_groups=[list(range(NC))],
                    ins=[cm_in[:]], outs=[cm_out[:]])
                return ag_in, ag_out, at_sb, cm_out

            def col_exp(t, at_sb, cm_out):
                """subtract global col-max, exp in place (bf16), colsum AR."""
                cmg = smp.tile([P, NKB], F32, tag="sm", name=f"cmg{t}")
                nc.sync.dma_start(cmg[:], cm_out[:])
                nc.vector.tensor_tensor(
                    at_sb[:], at_sb[:],
                    cmg[:, :, None].to_broadcast((P, NKB, RPC)), op=OP.subtract)
                nc.scalar.activation(at_sb[:], at_sb[:], ACT.Exp,
                                     bias=0.0, scale=SCALE)
                cs_loc = smp.tile([P, NKB], F32, tag="sm", name=f"csl{t}")
                nc.vector.reduce_sum(cs_loc[:], at_sb[:], axis=AX.X)
                cs_in = dram3.tile([P, NKB], F32, tag="csin", name=f"csin{t}")
                cs_out = dram3.tile([P, NKB], F32, tag="csout", addr_space="Shared",
                                    name=f"csout{t}")
                nc.sync.dma_start(cs_in[:], cs_loc[:])
                nc.gpsimd.collective_compute(
                    "AllReduce", OP.add, replica_groups=[list(range(NC))],
                    ins=[cs_in[:]], outs=[cs_out[:]])
                return cs_out

            def col_pack(t, ag_in, ag_out, at_sb, cs_out):
                """normalize by 1/colsum -> fp8 payload -> AllGather."""
                csg = smp.tile([P, NKB], F32, tag="sm", name=f"csg{t}")
                nc.sync.dma_start(csg[:], cs_out[:])
                csinv = smp.tile([P, NKB], F32, tag="sm", name=f"csi{t}")
                nc.vector.reciprocal(csinv[:], csg[:])
                for h in range(4):
                    stg = stgp.tile([P, 8, RPC], FP8, tag="stg", name=f"stg{t}_{h}")
                    nc.vector.tensor_tensor(
                        stg[:], at_sb[:, 8 * h:8 * h + 8, :],
                        csinv[:, 8 * h:8 * h + 8, None]
                        .to_broadcast((P, 8, RPC)), op=OP.mult)
                    nc.sync.dma_start(
                        ag_in[h * 8 * P:(h + 1) * 8 * P, :]
                        .rearrange("(o p) n -> p o n", p=P), stg[:])
                nc.gpsimd.collective_compute(
                    "AllGather", OP.bypass, replica_groups=[list(range(NC))],
                    ins=[ag_in[:]], outs=[ag_out[:]])

            def row_side(t, chunk):
                """row softmax in place (bf16), then transpose -> pt fp8."""
                for rt in range(NRT):
                    rm = smp.tile([P, 1], F32, tag="sm", name=f"rm_{t}_{rt}")
                    nc.vector.reduce_max(rm[:], chunk[rt][:], axis=AX.X)
                    bias = smp.tile([P, 1], F32, tag="sm", name=f"bias_{t}_{rt}")
                    nc.vector.tensor_scalar_mul(bias[:], rm[:], -SCALE)
                    ssum = smp.tile([P, 1], F32, tag="sm", name=f"ss_{t}_{rt}")
                    nc.scalar.activation(chunk[rt][:], chunk[rt][:], ACT.Exp,
                                         bias=bias[:], scale=SCALE,
                                         accum_out=ssum[:])
                    rs = smp.tile([P, 1], F32, tag="sm", name=f"rs_{t}_{rt}")
                    nc.vector.reciprocal(rs[:], ssum[:])
                    nc.scalar.activation(chunk[rt][:], chunk[rt][:], ACT.Copy,
                                         bias=0.0, scale=rs[:])
                pt = ptp.tile([P, NKB, RPC], FP8, tag="pt", name=f"pt{t}")
                transpose_quarters(t, chunk, pt, "r")
                return pt

            def m_phase(u, pt, ag_out):
                """M^T tiles = (S12_hat @ S21_hat)^T[jtile, R_c]; stats to out[u].

                DoubleRow fp8: stationary [128, 2, 128] from the gathered
                S21 payload (k-major), moving pt [128, 2, 512].
                """
                racc = stp.tile([P, RPC], F32, tag="racc", name=f"racc{u}")
                nc.vector.memset(racc[:], 0.0)
                dvallw = stp.tile([P, NRT, 32], F32, tag="dvall", name=f"dvall{u}")
                cm32 = stp.tile([P, 32], F32, tag="cm32", name=f"cm32{u}")
                nwselJ = nwsel.rearrange("p (j b) -> p j b", b=NRT)
                for jg in range(NC):
                    pss = [psM.tile([P, 512], F32, tag="psM", name=f"mps{u}_{jg}_{j2}")
                           for j2 in range(4)]
                    for kb2 in range(NKB // 2):
                        qsb = qsbp.tile([P, 2, RPC], FP8, tag="qsb",
                                        name=f"qs{u}_{jg}_{kb2}")
                        nc.sync.dma_start(
                            qsb[:], ag_out[jg * NTOK + kb2 * 2 * P:
                                           jg * NTOK + (kb2 + 1) * 2 * P, :]
                            .rearrange("(o p) n -> p o n", p=P))
                        for j2 in range(4):
                            nc.tensor.matmul(
                                pss[j2][:], qsb[:, :, j2 * P:(j2 + 1) * P],
                                pt[:, 2 * kb2:2 * kb2 + 2, :],
                                start=(kb2 == 0), stop=(kb2 == NKB // 2 - 1),
                                perf_mode=DR)
                    for j2 in range(4):
                        j = 4 * jg + j2
                        msb = stp.tile([P, 512], F32, tag="msb", name=f"msb{u}_{j}")
                        nc.scalar.copy(msb[:], pss[j2][:])
                        tmp4 = stp.tile([P, 512], F32, tag="tmp4", name=f"t4_{u}_{j}")
                        nc.vector.tensor_tensor(tmp4[:], msb[:], imask4[:], op=OP.mult)
                        dv4 = smp.tile([P, NRT], F32, tag="sm4", name=f"dv4_{u}_{j}")
                        nc.vector.reduce_sum(
                            dv4[:], tmp4.rearrange("p (b q) -> p b q", q=P), axis=AX.X)
                        dv4w = smp.tile([P, NRT], F32, tag="sm4", name=f"dvw_{u}_{j}")
                        nc.vector.tensor_tensor(dv4w[:], dv4[:], nwselJ[:, j, :],
                                                op=OP.mult)
                        nc.vector.tensor_copy(dvallw[:, :, j], dv4w[:])
                        sc = stp.tile([P, NRT, P], F32, tag="tmp4", name=f"sc_{u}_{j}")
                        nc.vector.tensor_tensor(
                            sc[:], imask4[:],
                            dv4w[:, :, None].to_broadcast((P, NRT, P)), op=OP.mult)
                        nc.vector.tensor_add(
                            msb.rearrange("p (b q) -> p b q", q=P), msb.rearrange(
                                "p (b q) -> p b q", q=P), sc[:])
                        nc.vector.reduce_max(cm32[:, j:j + 1], msb[:], axis=AX.X)
                        nc.vector.tensor_tensor(racc[:], racc[:], msb[:], op=OP.max)
                # diag output = -sum_j dvallw
                dsum = smp.tile([P, NRT], F32, tag="sm4", name=f"dsum{u}")
                nc.vector.reduce_sum(dsum[:], dvallw[:], axis=AX.X)
                diag = smp.tile([P, NRT], F32, tag="sm4", name=f"diag{u}")
                nc.vector.tensor_scalar_mul(diag[:], dsum[:], -1.0)
                nc.sync.dma_start(out[u, :, 0:RPC], racc[:])
                nc.sync.dma_start(out[u, :, RPC:RPC + 32], cm32[:])
                nc.sync.dma_start(out[u, :, RPC + 32:OUT_W], diag[:])

            # ---------------- main pipeline (2-term m_phase lag) ----------------
            pending = []   # [(u, pt, ag_out), ...]
            for t, (is_tri, gk, la, ra) in enumerate(TERMS):
                if is_tri:
                    lhs_a = compute_ut(gk, la, f"a{t}")
                else:
                    lhs_a = load_x(la, f"a{t}")
                chunk = side_chunk(t, lhs_a, ra)
                ag_in, ag_out, at_sb, cm_out = col_side(t, chunk)
                if len(pending) >= 2:
                    m_phase(*pending.pop(0))
                cs_out = col_exp(t, at_sb, cm_out)
                pt = row_side(t, chunk)
                col_pack(t, ag_in, ag_out, at_sb, cs_out)
                if t in (1, 2, 3):
                    kick_gram_ar(3 - t)
                pending.append((t, pt, ag_out))
            for args in pending:
                m_phase(*args)

    nc.finalize()
    return nc


_PROGRAM = None


def _get_program():
    global _PROGRAM
    if _PROGRAM is None:
        _PROGRAM = build_program()
    return _PROGRAM


def _normalize(x):
    n = np.linalg.norm(x.astype(np.float32), axis=-1, keepdims=True)
    return (x / np.maximum(n, 1e-12)).astype(np.float32)


def _build_in_maps(inputs):
    nf = [_normalize(np.asarray(inputs[k], np.float32))
          for k in ("feat0", "feat1", "feat2")]
    nfT = [np.ascontiguousarray(x.T) for x in nf]

    in_maps = []
    for c in range(NC):
        rows = slice(c * RPC, (c + 1) * RPC)
        m = {}
        for i in range(3):
            m[f"x{i}"] = np.ascontiguousarray(nfT[i][:, rows])
            m[f"w{i}"] = np.ascontiguousarray(nf[i][rows])
            m[f"f{i}"] = nfT[i]
        wsel = np.zeros((P, P), np.float32)
        for b in range(NRT):
            j = 4 * c + b
            wsel[:, 4 * j + b] = 1.0     # wselJ[p, j, b] layout
        m["wsel"] = wsel
        in_maps.append(m)
    return in_maps


def _reduce(results):
    """results: list (per core) of {'out': [6, 128, OUT_W]} -> scalar loss."""
    L = np.zeros(6, np.float64)
    for t in range(6):
        rowpart = 0.0
        colmax = np.full(NTOK, -np.inf)
        diag_g = np.zeros(NTOK)
        for c in range(NC):
            o = results[c]["out"][t].astype(np.float64)
            racc = o[:, 0:RPC]
            cm32 = o[:, RPC:RPC + 32]
            dacc = o[:, RPC + 32:OUT_W]
            rowmax_local = racc.max(axis=0)                   # [512]
            diag_local = dacc.T.reshape(RPC)                  # [512]
            rowpart += np.maximum(rowmax_local + MARGIN - diag_local, 0.0).sum()
            colmax = np.maximum(colmax, cm32.T.reshape(NTOK))
            diag_g[c * RPC:(c + 1) * RPC] = diag_local
        colpart = np.maximum(colmax + MARGIN - diag_g, 0.0).sum()
        L[t] = (rowpart + colpart) / (2.0 * NTOK)
    loss = (L[0] + L[1] + L[2]) / 3.0 + (L[3] + L[4] + L[5]) / 3.0
    return np.float32(loss)


def kernel(feat0, feat1, feat2):
    in_maps = _build_in_maps({"feat0": feat0, "feat1": feat1, "feat2": feat2})
    nc = _get_program()
    res = run_bass_kernel_spmd(nc, in_maps, core_ids=list(range(NC)))
    return _reduce(res.results)


if __name__ == "__main__":
    rng = np.random.default_rng(0)
    f0 = rng.standard_normal((NTOK, D), dtype=np.float32)
    f1 = rng.standard_normal((NTOK, D), dtype=np.float32)
    f2 = rng.standard_normal((NTOK, D), dtype=np.float32)
    print("loss:", kernel(f0, f1, f2))


# revision 20
# speedup vs baseline: 1.1205x; 1.1205x over previous
"""Trainium2 Bass kernel for the pairwise+triplewise cycle-consistency loss.

Strategy (8 NeuronCores, tensor-parallel over rows of each [N,N] block):
  - All six cycle-term matrices have the form  A = U @ nf_j^T  with
    U = nf_i (pairs) or U = nf_i @ G_k (triples), G_k = nf_k^T nf_k [D,D],
    collapsing the [N,N]@[N,N] triple products into [D,D] Gram matmuls.
  - Each core owns a 512-row block R_c and computes A[R_c,:] ONCE (f32r
    matmuls, bf16 result). S12_hat rows come from a local row-softmax.
    S21_hat (the column softmax) is derived from PE-transposed A tiles
    plus two tiny [128,32] AllReduces (col-max, col-sum) — the second
    [RPC,D]@[D,N] matmul set of the baseline is gone.
  - Both M-product operands are quantized to fp8e4 (values in [0,1]);
    S21_hat columns are AllGathered as a [N, RPC] fp8 payload and the
    M^T column-tiles are computed with DoubleRow fp8 matmuls (2x rate),
    accumulating rowmax/colmax/diag stats on the fly. Host assembles the
    scalar loss.
"""
import sys
sys.path.insert(0, "/opt/trn_rl_repo")

import math
import numpy as np

import concourse.bass as bass
import concourse.mybir as mybir
import concourse.tile as tile
from concourse import bacc
from concourse.bass_utils import run_bass_kernel_spmd
from concourse.masks import make_identity

F32 = mybir.dt.float32
F32R = mybir.dt.float32r
BF16 = mybir.dt.bfloat16
FP8 = mybir.dt.float8e4
AX = mybir.AxisListType
OP = mybir.AluOpType
ACT = mybir.ActivationFunctionType
DR = mybir.MatmulPerfMode.DoubleRow

NTOK = 4096          # rows per view
D = 1024             # feature dim
NC = 8               # cores
RPC = NTOK // NC     # rows per core (512)
P = 128
NRT = RPC // P       # rowtiles per core (4)
NS = 8               # 512-col strips of A
DKB = D // P         # d-blocks (8)
NKB = NTOK // P      # k-tiles (32)
SCALE = math.log(NTOK) / 0.1
MARGIN = 0.5

# term table: (is_tri, gram_idx, lhsA, rhsA); lhs indexes x_i, rhs indexes f_i.
# For tri terms lhs is G[gram_idx] @ x_i.
TERMS = [
    (False, None, 0, 1),   # S01
    (False, None, 0, 2),   # S02
    (False, None, 1, 2),   # S12
    (True, 2, 0, 1),       # S02 @ S21 = nf0 G2 nf1^T
    (True, 1, 0, 2),       # S01 @ S12 = nf0 G1 nf2^T
    (True, 0, 1, 2),       # S10 @ S02 = nf1 G0 nf2^T
]

OUT_W = RPC + 32 + NRT   # racc 512 | colmax32 32 | diag 4


def build_program():
    nc = bacc.Bacc("TRN2", target_bir_lowering=False, debug=False, num_devices=NC)

    xs = [nc.dram_tensor(f"x{i}", [D, RPC], F32R, kind="ExternalInput") for i in range(3)]
    xbs = [nc.dram_tensor(f"xb{i}", [D, RPC], BF16, kind="ExternalInput") for i in range(3)]
    ws = [nc.dram_tensor(f"w{i}", [RPC, D], F32R, kind="ExternalInput") for i in range(3)]
    fs = [nc.dram_tensor(f"f{i}", [D, NTOK], BF16, kind="ExternalInput") for i in range(3)]
    wsel_in = nc.dram_tensor("wsel", [P, P], F32, kind="ExternalInput")
    out = nc.dram_tensor("out", [6, P, OUT_W], F32, kind="ExternalOutput")

    with tile.TileContext(nc) as tc:
        with (
            tc.tile_pool(name="cst", bufs=1) as cst,
            tc.tile_pool(name="lhs", bufs=2) as lhsp,
            tc.tile_pool(name="rhs", bufs=2) as rhsp,
            tc.tile_pool(name="abf", bufs=4) as abfp,
            tc.tile_pool(name="at", bufs=1) as atp,
            tc.tile_pool(name="pt", bufs=3) as ptp,
            tc.tile_pool(name="stg", bufs=2) as stgp,
            tc.tile_pool(name="qsb", bufs=3) as qsbp,
            tc.tile_pool(name="st", bufs=2) as stp,
            tc.tile_pool(name="sm", bufs=4) as smp,
            tc.tile_pool(name="psA", bufs=2, space="PSUM") as psA,
            tc.tile_pool(name="psT", bufs=2, space="PSUM") as psT,
            tc.tile_pool(name="psM", bufs=4, space="PSUM") as psM,
            tc.tile_pool(name="dram", bufs=1, space="DRAM") as dram,
            tc.tile_pool(name="dram2", bufs=3, space="DRAM") as dram2,
            tc.tile_pool(name="dram3", bufs=2, space="DRAM") as dram3,
        ):
            # constants
            identb = cst.tile([P, P], BF16)
            make_identity(nc, identb)
            wsel = cst.tile([P, P], F32)
            nc.sync.dma_start(wsel[:], wsel_in[:])
            nwsel = cst.tile([P, P], F32)
            nc.vector.tensor_scalar_mul(nwsel[:], wsel[:], -1.0)
            # imask4[p, 128b+p] = 1 for b in 0..3 (diag candidate positions)
            identf = cst.tile([P, P], F32)
            make_identity(nc, identf)
            imask4 = cst.tile([P, NRT, P], F32)
            for b in range(NRT):
                nc.vector.tensor_copy(imask4[:, b, :], identf[:])

            # ---------------- Gram phase ----------------
            gins = [dram.tile([D, D], F32, tag=f"gin{k}", name=f"gin{k}")
                    for k in range(3)]
            gouts = [dram.tile([D, D], F32, tag=f"gout{k}", addr_space="Shared",
                               name=f"gout{k}") for k in range(3)]
            for k in range(3):
                w_sb = lhsp.tile([P, NRT, D], F32R, tag="lhs", name=f"w_sb{k}")
                nc.sync.dma_start(w_sb[:], ws[k].rearrange("(o p) d -> p o d", p=P))
                for d1 in range(DKB):
                    for d2 in range(2):
                        ps = psA.tile([P, 512], F32, tag="psA", name=f"gps{k}_{d1}_{d2}")
                        for nt in range(NRT):
                            nc.tensor.matmul(
                                ps[:], w_sb[:, nt, d1 * P:(d1 + 1) * P],
                                w_sb[:, nt, d2 * 512:(d2 + 1) * 512],
                                start=(nt == 0), stop=(nt == NRT - 1))
                        gtmp = stp.tile([P, 512], F32, tag="msb", name=f"gt{k}_{d1}_{d2}")
                        nc.scalar.copy(gtmp[:], ps[:])
                        nc.sync.dma_start(
                            gins[k][d1 * P:(d1 + 1) * P,
                                    d2 * 512:(d2 + 1) * 512], gtmp[:])

            def kick_gram_ar(k):
                nc.gpsimd.collective_compute(
                    "AllReduce", OP.add, replica_groups=[list(range(NC))],
                    ins=[gins[k][:]], outs=[gouts[k][:]])

            # ---------------- helpers ----------------
            def load_xb(i, nm):
                t = lhsp.tile([P, DKB, RPC], BF16, tag="lhs", name=f"xb_{nm}")
                nc.sync.dma_start(t[:], xbs[i].rearrange("(o p) r -> p o r", p=P))
                return t

            def compute_ut(gk, i, nm):
                """U^T[:, R_c] = G_k @ x_i  -> [128, DKB, RPC] bf16 tile."""
                x_sb = lhsp.tile([P, DKB, RPC], F32R, tag="lhs", name=f"utx_{nm}")
                nc.sync.dma_start(x_sb[:], xs[i].rearrange("(o p) r -> p o r", p=P))
                ut = lhsp.tile([P, DKB, RPC], BF16, tag="lhs", name=f"ut_{nm}")
                for grp in range(2):
                    pss = [psM.tile([P, 512], F32, tag="psM", name=f"utps_{nm}_{grp}_{d4}")
                           for d4 in range(4)]
                    for half in range(2):
                        gh = rhsp.tile([P, 4, D], F32R, tag="rhs", name=f"gh_{nm}_{grp}_{half}")
                        nc.sync.dma_start(
                            gh[:], gouts[gk][half * 512:(half + 1) * 512]
                            .rearrange("(o p) d -> p o d", p=P).bitcast(F32R))
                        for d4 in range(4):
                            dp = 4 * grp + d4
                            for db in range(4):
                                nc.tensor.matmul(
                                    pss[d4][:], gh[:, db, dp * P:(dp + 1) * P],
                                    x_sb[:, 4 * half + db, :],
                                    start=(half == 0 and db == 0),
                                    stop=(half == 1 and db == 3))
                    for d4 in range(4):
                        nc.scalar.copy(ut[:, 4 * grp + d4, :], pss[d4][:])
                return ut

            def side_chunk(t, lhs_t, fj):
                """A[R_c, :] raw logits (pre-scale) as 4 bf16 quarter tiles."""
                chunk = [abfp.tile([P, NTOK], BF16, tag="abf", name=f"ch_{t}_{rt}")
                         for rt in range(NRT)]
                for s in range(NS):
                    rsb = rhsp.tile([P, DKB, 512], BF16, tag="rhs", name=f"rs_{t}_{s}")
                    nc.sync.dma_start(
                        rsb[:], fs[fj][:, s * 512:(s + 1) * 512]
                        .rearrange("(o p) n -> p o n", p=P))
                    for rt in range(NRT):
                        ps = psA.tile([P, 512], F32, tag="psA", name=f"aps_{t}_{s}_{rt}")
                        for kb in range(DKB):
                            nc.tensor.matmul(
                                ps[:], lhs_t[:, kb, rt * P:(rt + 1) * P],
                                rsb[:, kb, :], start=(kb == 0), stop=(kb == DKB - 1))
                        nc.scalar.copy(chunk[rt][:, s * 512:(s + 1) * 512], ps[:])
                return chunk

            def transpose_quarters(t, chunk, dst, nm):
                """PE-transpose chunk[rt] (4x [P, NTOK] bf16) into dst [P, NKB, RPC]."""
                for rt in range(NRT):
                    for g in range(NKB // 4):
                        tp = psT.tile([P, 512], BF16, tag="psT", name=f"tp{nm}_{t}_{rt}_{g}")
                        for q in range(4):
                            kb = 4 * g + q
                            nc.tensor.transpose(
                                tp[:, q * P:(q + 1) * P],
                                chunk[rt][:, kb * P:(kb + 1) * P], identb[:])
                        nc.vector.tensor_copy(
                            dst[:, 4 * g:4 * g + 4, rt * P:(rt + 1) * P],
                            tp.rearrange("p (o q) -> p o q", q=P))

            def col_side(t, chunk):
                """S21_hat columns [NTOK, R_c] -> fp8 allgather payload."""
                ag_in = dram2.tile([NTOK, RPC], FP8, tag="agin", name=f"agin{t}")
                ag_out = dram2.tile([NC * NTOK, RPC], FP8, tag="agout",
                                    addr_space="Shared", name=f"agout{t}")
                at_sb = atp.tile([P, NKB, RPC], BF16, tag="at", name=f"at{t}")
                transpose_quarters(t, chunk, at_sb, "c")
                # local col-max -> AllReduce max
                cm_loc = smp.tile([P, NKB], F32, tag="sm", name=f"cml{t}")
                nc.vector.reduce_max(cm_loc[:], at_sb[:], axis=AX.X)
                cm_in = dram3.tile([P, NKB], F32, tag="cmin", name=f"cmin{t}")
                cm_out = dram3.tile([P, NKB], F32, tag="cmout", addr_space="Shared",
                                    name=f"cmout{t}")
                nc.sync.dma_start(cm_in[:], cm_loc[:])
                nc.gpsimd.collective_compute(
                    "AllReduce", OP.max, replica_groups=[list(range(NC))],
                    ins=[cm_in[:]], outs=[cm_out[:]])
                return ag_in, ag_out, at_sb, cm_loc, cm_out

            def col_exp(t, at_sb, cm_loc):
                """exp in place against LOCAL col-max (collective-free), and
                local col-sums. Global correction happens in col_pack."""
                nc.vector.tensor_tensor(
                    at_sb[:], at_sb[:],
                    cm_loc[:, :, None].to_broadcast((P, NKB, RPC)), op=OP.subtract)
                nc.scalar.activation(at_sb[:], at_sb[:], ACT.Exp,
                                     bias=0.0, scale=SCALE)
                ls_loc = smp.tile([P, NKB], F32, tag="sm", name=f"lsl{t}")
                nc.vector.reduce_sum(ls_loc[:], at_sb[:], axis=AX.X)
                return ls_loc

            def col_correct(t, cm_loc, cm_out, ls_loc):
                """g = exp(S*(cm_loc - cm_glob)); AR-add of g*lsum."""
                cmg = smp.tile([P, NKB], F32, tag="sm", name=f"cmg{t}")
                nc.sync.dma_start(cmg[:], cm_out[:])
                gcor = smp.tile([P, NKB], F32, tag="sm", name=f"gc{t}")
                nc.vector.tensor_tensor(gcor[:], cm_loc[:], cmg[:], op=OP.subtract)
                nc.scalar.activation(gcor[:], gcor[:], ACT.Exp, bias=0.0,
                                     scale=SCALE)
                gls = smp.tile([P, NKB], F32, tag="sm", name=f"gls{t}")
                nc.vector.tensor_tensor(gls[:], gcor[:], ls_loc[:], op=OP.mult)
                cs_in = dram3.tile([P, NKB], F32, tag="csin", name=f"csin{t}")
                cs_out = dram3.tile([P, NKB], F32, tag="csout", addr_space="Shared",
                                    name=f"csout{t}")
                nc.sync.dma_start(cs_in[:], gls[:])
                nc.gpsimd.collective_compute(
                    "AllReduce", OP.add, replica_groups=[list(range(NC))],
                    ins=[cs_in[:]], outs=[cs_out[:]])
                return gcor, cs_out

            def col_pack(t, ag_in, ag_out, at_sb, gcor, cs_out):
                """payload = at_sb * (g/CS_glob) -> fp8 -> AllGather."""
                csg = smp.tile([P, NKB], F32, tag="sm", name=f"csg{t}")
                nc.sync.dma_start(csg[:], cs_out[:])
                csinv = smp.tile([P, NKB], F32, tag="sm", name=f"csi{t}")
                nc.vector.reciprocal(csinv[:], csg[:])
                fac = smp.tile([P, NKB], F32, tag="sm", name=f"fac{t}")
                nc.vector.tensor_tensor(fac[:], gcor[:], csinv[:], op=OP.mult)
                for h in range(4):
                    stg = stgp.tile([P, 8, RPC], FP8, tag="stg", name=f"stg{t}_{h}")
                    nc.vector.tensor_tensor(
                        stg[:], at_sb[:, 8 * h:8 * h + 8, :],
                        fac[:, 8 * h:8 * h + 8, None]
                        .to_broadcast((P, 8, RPC)), op=OP.mult)
                    nc.sync.dma_start(
                        ag_in[h * 8 * P:(h + 1) * 8 * P, :]
                        .rearrange("(o p) n -> p o n", p=P), stg[:])
                nc.gpsimd.collective_compute(
                    "AllGather", OP.bypass, replica_groups=[list(range(NC))],
                    ins=[ag_in[:]], outs=[ag_out[:]])

            def row_side(t, chunk):
                """row softmax in place (bf16), then transpose -> pt fp8."""
                for rt in range(NRT):
                    rm = smp.tile([P, 1], F32, tag="sm", name=f"rm_{t}_{rt}")
                    nc.vector.reduce_max(rm[:], chunk[rt][:], axis=AX.X)
                    bias = smp.tile([P, 1], F32, tag="sm", name=f"bias_{t}_{rt}")
                    nc.vector.tensor_scalar_mul(bias[:], rm[:], -SCALE)
                    ssum = smp.tile([P, 1], F32, tag="sm", name=f"ss_{t}_{rt}")
                    nc.scalar.activation(chunk[rt][:], chunk[rt][:], ACT.Exp,
                                         bias=bias[:], scale=SCALE,
                                         accum_out=ssum[:])
                    rs = smp.tile([P, 1], F32, tag="sm", name=f"rs_{t}_{rt}")
                    nc.vector.reciprocal(rs[:], ssum[:])
                    nc.scalar.activation(chunk[rt][:], chunk[rt][:], ACT.Copy,
                                         bias=0.0, scale=rs[:])
                pt = ptp.tile([P, NKB, RPC], FP8, tag="pt", name=f"pt{t}")
                transpose_quarters(t, chunk, pt, "r")
                return pt

            def m_phase(u, pt, ag_out):
                """M^T tiles = (S12_hat @ S21_hat)^T[jtile, R_c]; stats to out[u].

                DoubleRow fp8: stationary [128, 2, 128] from the gathered
                S21 payload (k-major), moving pt [128, 2, 512].
                """
                racc = stp.tile([P, RPC], F32, tag="racc", name=f"racc{u}")
                nc.vector.memset(racc[:], 0.0)
                dvallw = stp.tile([P, NRT, 32], F32, tag="dvall", name=f"dvall{u}")
                cm32 = stp.tile([P, 32], F32, tag="cm32", name=f"cm32{u}")
                nwselJ = nwsel.rearrange("p (j b) -> p j b", b=NRT)
                for jg in range(NC):
                    pss = [psM.tile([P, 512], F32, tag="psM", name=f"mps{u}_{jg}_{j2}")
                           for j2 in range(4)]
                    for kb2 in range(NKB // 2):
                        qsb = qsbp.tile([P, 2, RPC], FP8, tag="qsb",
                                        name=f"qs{u}_{jg}_{kb2}")
                        nc.sync.dma_start(
                            qsb[:], ag_out[jg * NTOK + kb2 * 2 * P:
                                           jg * NTOK + (kb2 + 1) * 2 * P, :]
                            .rearrange("(o p) n -> p o n", p=P))
                        for j2 in range(4):
                            nc.tensor.matmul(
                                pss[j2][:], qsb[:, :, j2 * P:(j2 + 1) * P],
                                pt[:, 2 * kb2:2 * kb2 + 2, :],
                                start=(kb2 == 0), stop=(kb2 == NKB // 2 - 1),
                                perf_mode=DR)
                    for j2 in range(4):
                        j = 4 * jg + j2
                        msb = stp.tile([P, 512], F32, tag="msb", name=f"msb{u}_{j}")
                        nc.scalar.copy(msb[:], pss[j2][:])
                        tmp4 = stp.tile([P, 512], F32, tag="tmp4", name=f"t4_{u}_{j}")
                        nc.vector.tensor_tensor(tmp4[:], msb[:], imask4[:], op=OP.mult)
                        dv4 = smp.tile([P, NRT], F32, tag="sm4", name=f"dv4_{u}_{j}")
                        nc.vector.reduce_sum(
                            dv4[:], tmp4.rearrange("p (b q) -> p b q", q=P), axis=AX.X)
                        dv4w = smp.tile([P, NRT], F32, tag="sm4", name=f"dvw_{u}_{j}")
                        nc.vector.tensor_tensor(dv4w[:], dv4[:], nwselJ[:, j, :],
                                                op=OP.mult)
                        nc.vector.tensor_copy(dvallw[:, :, j], dv4w[:])
                        sc = stp.tile([P, NRT, P], F32, tag="tmp4", name=f"sc_{u}_{j}")
                        nc.vector.tensor_tensor(
                            sc[:], imask4[:],
                            dv4w[:, :, None].to_broadcast((P, NRT, P)), op=OP.mult)
                        nc.vector.tensor_add(
                            msb.rearrange("p (b q) -> p b q", q=P), msb.rearrange(
                                "p (b q) -> p b q", q=P), sc[:])
                        nc.vector.reduce_max(cm32[:, j:j + 1], msb[:], axis=AX.X)
                        nc.vector.tensor_tensor(racc[:], racc[:], msb[:], op=OP.max)
                # diag output = -sum_j dvallw
                dsum = smp.tile([P, NRT], F32, tag="sm4", name=f"dsum{u}")
                nc.vector.reduce_sum(dsum[:], dvallw[:], axis=AX.X)
                diag = smp.tile([P, NRT], F32, tag="sm4", name=f"diag{u}")
                nc.vector.tensor_scalar_mul(diag[:], dsum[:], -1.0)
                nc.sync.dma_start(out[u, :, 0:RPC], racc[:])
                nc.sync.dma_start(out[u, :, RPC:RPC + 32], cm32[:])
                nc.sync.dma_start(out[u, :, RPC + 32:OUT_W], diag[:])

            # ---------------- main pipeline (2-term m_phase lag) ----------------
            pending = []   # [(u, pt, ag_out), ...]
            for t, (is_tri, gk, la, ra) in enumerate(TERMS):
                if is_tri:
                    lhs_a = compute_ut(gk, la, f"a{t}")
                else:
                    lhs_a = load_xb(la, f"a{t}")
                chunk = side_chunk(t, lhs_a, ra)
                ag_in, ag_out, at_sb, cm_loc, cm_out = col_side(t, chunk)
                ls_loc = col_exp(t, at_sb, cm_loc)
                pt = row_side(t, chunk)
                if len(pending) >= 2:
                    m_phase(*pending.pop(0))
                gcor, cs_out = col_correct(t, cm_loc, cm_out, ls_loc)
                col_pack(t, ag_in, ag_out, at_sb, gcor, cs_out)
                if t in (1, 2, 3):
                    kick_gram_ar(3 - t)
                pending.append((t, pt, ag_out))
            for args in pending:
                m_phase(*args)

    nc.finalize()
    return nc


_PROGRAM = None


def _get_program():
    global _PROGRAM
    if _PROGRAM is None:
        _PROGRAM = build_program()
    return _PROGRAM


def _normalize(x):
    n = np.linalg.norm(x.astype(np.float32), axis=-1, keepdims=True)
    return (x / np.maximum(n, 1e-12)).astype(np.float32)


def _build_in_maps(inputs):
    nf = [_normalize(np.asarray(inputs[k], np.float32))
          for k in ("feat0", "feat1", "feat2")]
    nfT = [np.ascontiguousarray(x.T) for x in nf]

    import ml_dtypes
    nfTb = [x.astype(ml_dtypes.bfloat16) for x in nfT]
    in_maps = []
    for c in range(NC):
        rows = slice(c * RPC, (c + 1) * RPC)
        m = {}
        for i in range(3):
            m[f"x{i}"] = np.ascontiguousarray(nfT[i][:, rows])
            m[f"xb{i}"] = np.ascontiguousarray(nfTb[i][:, rows])
            m[f"w{i}"] = np.ascontiguousarray(nf[i][rows])
            m[f"f{i}"] = nfTb[i]
        wsel = np.zeros((P, P), np.float32)
        for b in range(NRT):
            j = 4 * c + b
            wsel[:, 4 * j + b] = 1.0     # wselJ[p, j, b] layout
        m["wsel"] = wsel
        in_maps.append(m)
    return in_maps


def _reduce(results):
    """results: list (per core) of {'out': [6, 128, OUT_W]} -> scalar loss."""
    L = np.zeros(6, np.float64)
    for t in range(6):
        rowpart = 0.0
        colmax = np.full(NTOK, -np.inf)
        diag_g = np.zeros(NTOK)
        for c in range(NC):
            o = results[c]["out"][t].astype(np.float64)
            racc = o[:, 0:RPC]
            cm32 = o[:, RPC:RPC + 32]
            dacc = o[:, RPC + 32:OUT_W]
            rowmax_local = racc.max(axis=0)                   # [512]
            diag_local = dacc.T.reshape(RPC)                  # [512]
            rowpart += np.maximum(rowmax_local + MARGIN - diag_local, 0.0).sum()
            colmax = np.maximum(colmax, cm32.T.reshape(NTOK))
            diag_g[c * RPC:(c + 1) * RPC] = diag_local
        colpart = np.maximum(colmax + MARGIN - diag_g, 0.0).sum()
        L[t] = (rowpart + colpart) / (2.0 * NTOK)
    loss = (L[0] + L[1] + L[2]) / 3.0 + (L[3] + L[4] + L[5]) / 3.0
    return np.float32(loss)


def kernel(feat0, feat1, feat2):
    in_maps = _build_in_maps({"feat0": feat0, "feat1": feat1, "feat2": feat2})
    nc = _get_program()
    res = run_bass_kernel_spmd(nc, in_maps, core_ids=list(range(NC)))
    return _reduce(res.results)


if __name__ == "__main__":
    rng = np.random.default_rng(0)
    f0 = rng.standard_normal((NTOK, D), dtype=np.float32)
    f1 = rng.standard_normal((NTOK, D), dtype=np.float32)
    f2 = rng.standard_normal((NTOK, D), dtype=np.float32)
    print("loss:", kernel(f0, f1, f2))


# revision 25
# speedup vs baseline: 1.1514x; 1.0276x over previous
"""Trainium2 Bass kernel for the pairwise+triplewise cycle-consistency loss.

Strategy (8 NeuronCores, tensor-parallel over rows of each [N,N] block):
  - All six cycle-term matrices have the form  A = U @ nf_j^T  with
    U = nf_i (pairs) or U = nf_i @ G_k (triples), G_k = nf_k^T nf_k [D,D],
    collapsing the [N,N]@[N,N] triple products into [D,D] Gram matmuls.
  - Each core owns a 512-row block R_c and computes A[R_c,:] ONCE (f32r
    matmuls, bf16 result). S12_hat rows come from a local row-softmax.
    S21_hat (the column softmax) is derived from PE-transposed A tiles
    plus two tiny [128,32] AllReduces (col-max, col-sum) — the second
    [RPC,D]@[D,N] matmul set of the baseline is gone.
  - Both M-product operands are quantized to fp8e4 (values in [0,1]);
    S21_hat columns are AllGathered as a [N, RPC] fp8 payload and the
    M^T column-tiles are computed with DoubleRow fp8 matmuls (2x rate),
    accumulating rowmax/colmax/diag stats on the fly. Host assembles the
    scalar loss.
"""
import sys
sys.path.insert(0, "/opt/trn_rl_repo")

import math
import numpy as np

import concourse.bass as bass
import concourse.mybir as mybir
import concourse.tile as tile
from concourse import bacc
from concourse.bass_utils import run_bass_kernel_spmd
from concourse.masks import make_identity

F32 = mybir.dt.float32
F32R = mybir.dt.float32r
BF16 = mybir.dt.bfloat16
FP8 = mybir.dt.float8e4
AX = mybir.AxisListType
OP = mybir.AluOpType
ACT = mybir.ActivationFunctionType
DR = mybir.MatmulPerfMode.DoubleRow

NTOK = 4096          # rows per view
D = 1024             # feature dim
NC = 8               # cores
RPC = NTOK // NC     # rows per core (512)
P = 128
NRT = RPC // P       # rowtiles per core (4)
NS = 8               # 512-col strips of A
DKB = D // P         # d-blocks (8)
NKB = NTOK // P      # k-tiles (32)
SCALE = math.log(NTOK) / 0.1
MARGIN = 0.5

# term table: (is_tri, gram_idx, lhsA, rhsA); lhs indexes x_i, rhs indexes f_i.
# For tri terms lhs is G[gram_idx] @ x_i.
TERMS = [
    (False, None, 0, 1),   # S01
    (False, None, 0, 2),   # S02
    (False, None, 1, 2),   # S12
    (True, 2, 0, 1),       # S02 @ S21 = nf0 G2 nf1^T
    (True, 1, 0, 2),       # S01 @ S12 = nf0 G1 nf2^T
    (True, 0, 1, 2),       # S10 @ S02 = nf1 G0 nf2^T
]

OUT_W = RPC + 32 + NRT   # racc 512 | colmax32 32 | diag 4


def build_program():
    nc = bacc.Bacc("TRN2", target_bir_lowering=False, debug=False, num_devices=NC)

    xs = [nc.dram_tensor(f"x{i}", [D, RPC], F32R, kind="ExternalInput") for i in range(3)]
    xbs = [nc.dram_tensor(f"xb{i}", [D, RPC], BF16, kind="ExternalInput") for i in range(3)]
    ws = [nc.dram_tensor(f"w{i}", [RPC, D], F32R, kind="ExternalInput") for i in range(3)]
    fs = [nc.dram_tensor(f"f{i}", [D, NTOK], BF16, kind="ExternalInput") for i in range(3)]
    wsel_in = nc.dram_tensor("wsel", [P, P], F32, kind="ExternalInput")
    out = nc.dram_tensor("out", [6, P, OUT_W], F32, kind="ExternalOutput")

    with tile.TileContext(nc) as tc:
        with (
            tc.tile_pool(name="cst", bufs=1) as cst,
            tc.tile_pool(name="lhs", bufs=2) as lhsp,
            tc.tile_pool(name="rhs", bufs=2) as rhsp,
            tc.tile_pool(name="abf", bufs=4) as abfp,
            tc.tile_pool(name="at", bufs=1) as atp,
            tc.tile_pool(name="pt", bufs=3) as ptp,
            tc.tile_pool(name="stg", bufs=2) as stgp,
            tc.tile_pool(name="qsb", bufs=3) as qsbp,
            tc.tile_pool(name="st", bufs=2) as stp,
            tc.tile_pool(name="sm", bufs=4) as smp,
            tc.tile_pool(name="psA", bufs=2, space="PSUM") as psA,
            tc.tile_pool(name="psT", bufs=2, space="PSUM") as psT,
            tc.tile_pool(name="psM", bufs=4, space="PSUM") as psM,
            tc.tile_pool(name="dram", bufs=1, space="DRAM") as dram,
            tc.tile_pool(name="dram2", bufs=3, space="DRAM") as dram2,
            tc.tile_pool(name="dram3", bufs=2, space="DRAM") as dram3,
        ):
            # constants
            identb = cst.tile([P, P], BF16)
            make_identity(nc, identb)
            wsel = cst.tile([P, P], F32)
            nc.sync.dma_start(wsel[:], wsel_in[:])
            nwsel = cst.tile([P, P], F32)
            nc.vector.tensor_scalar_mul(nwsel[:], wsel[:], -1.0)
            # imask4[p, 128b+p] = 1 for b in 0..3 (diag candidate positions)
            identf = cst.tile([P, P], F32)
            make_identity(nc, identf)
            imask4 = cst.tile([P, NRT, P], F32)
            for b in range(NRT):
                nc.vector.tensor_copy(imask4[:, b, :], identf[:])

            # ---------------- Gram phase ----------------
            gins = [dram.tile([D, D], F32, tag=f"gin{k}", name=f"gin{k}")
                    for k in range(3)]
            gouts = [dram.tile([D, D], F32, tag=f"gout{k}", addr_space="Shared",
                               name=f"gout{k}") for k in range(3)]
            for k in range(3):
                w_sb = lhsp.tile([P, NRT, D], F32R, tag="lhs", name=f"w_sb{k}")
                nc.sync.dma_start(w_sb[:], ws[k].rearrange("(o p) d -> p o d", p=P))
                for d1 in range(DKB):
                    for d2 in range(2):
                        ps = psA.tile([P, 512], F32, tag="psA", name=f"gps{k}_{d1}_{d2}")
                        for nt in range(NRT):
                            nc.tensor.matmul(
                                ps[:], w_sb[:, nt, d1 * P:(d1 + 1) * P],
                                w_sb[:, nt, d2 * 512:(d2 + 1) * 512],
                                start=(nt == 0), stop=(nt == NRT - 1))
                        gtmp = stp.tile([P, 512], F32, tag="msb", name=f"gt{k}_{d1}_{d2}")
                        nc.scalar.copy(gtmp[:], ps[:])
                        nc.sync.dma_start(
                            gins[k][d1 * P:(d1 + 1) * P,
                                    d2 * 512:(d2 + 1) * 512], gtmp[:])

            def kick_gram_ar(k):
                nc.gpsimd.collective_compute(
                    "AllReduce", OP.add, replica_groups=[list(range(NC))],
                    ins=[gins[k][:]], outs=[gouts[k][:]])

            # ---------------- helpers ----------------
            def load_xb(i, nm):
                t = lhsp.tile([P, DKB, RPC], BF16, tag="lhs", name=f"xb_{nm}")
                nc.sync.dma_start(t[:], xbs[i].rearrange("(o p) r -> p o r", p=P))
                return t

            def compute_ut(gk, i, nm):
                """U^T[:, R_c] = G_k @ x_i  -> [128, DKB, RPC] bf16 tile."""
                x_sb = lhsp.tile([P, DKB, RPC], F32R, tag="lhs", name=f"utx_{nm}")
                nc.sync.dma_start(x_sb[:], xs[i].rearrange("(o p) r -> p o r", p=P))
                ut = lhsp.tile([P, DKB, RPC], BF16, tag="lhs", name=f"ut_{nm}")
                for grp in range(2):
                    pss = [psM.tile([P, 512], F32, tag="psM", name=f"utps_{nm}_{grp}_{d4}")
                           for d4 in range(4)]
                    for half in range(2):
                        gh = rhsp.tile([P, 4, D], F32R, tag="rhs", name=f"gh_{nm}_{grp}_{half}")
                        nc.sync.dma_start(
                            gh[:], gouts[gk][half * 512:(half + 1) * 512]
                            .rearrange("(o p) d -> p o d", p=P).bitcast(F32R))
                        for d4 in range(4):
                            dp = 4 * grp + d4
                            for db in range(4):
                                nc.tensor.matmul(
                                    pss[d4][:], gh[:, db, dp * P:(dp + 1) * P],
                                    x_sb[:, 4 * half + db, :],
                                    start=(half == 0 and db == 0),
                                    stop=(half == 1 and db == 3))
                    for d4 in range(4):
                        nc.scalar.copy(ut[:, 4 * grp + d4, :], pss[d4][:])
                return ut

            def side_chunk(t, lhs_t, fj):
                """A[R_c, :] raw logits (pre-scale) as 4 bf16 quarter tiles."""
                chunk = [abfp.tile([P, NTOK], BF16, tag="abf", name=f"ch_{t}_{rt}")
                         for rt in range(NRT)]
                for s in range(NS):
                    rsb = rhsp.tile([P, DKB, 512], BF16, tag="rhs", name=f"rs_{t}_{s}")
                    nc.sync.dma_start(
                        rsb[:], fs[fj][:, s * 512:(s + 1) * 512]
                        .rearrange("(o p) n -> p o n", p=P))
                    for rt in range(NRT):
                        ps = psA.tile([P, 512], F32, tag="psA", name=f"aps_{t}_{s}_{rt}")
                        for kb in range(DKB):
                            nc.tensor.matmul(
                                ps[:], lhs_t[:, kb, rt * P:(rt + 1) * P],
                                rsb[:, kb, :], start=(kb == 0), stop=(kb == DKB - 1))
                        nc.scalar.copy(chunk[rt][:, s * 512:(s + 1) * 512], ps[:])
                return chunk

            def transpose_quarters(t, chunk, dst, nm, eng=None):
                """PE-transpose chunk[rt] (4x [P, NTOK] bf16) into dst [P, NKB, RPC]."""
                eng = eng or nc.vector
                for rt in range(NRT):
                    for g in range(NKB // 4):
                        tp = psT.tile([P, 512], BF16, tag="psT", name=f"tp{nm}_{t}_{rt}_{g}")
                        for q in range(4):
                            kb = 4 * g + q
                            nc.tensor.transpose(
                                tp[:, q * P:(q + 1) * P],
                                chunk[rt][:, kb * P:(kb + 1) * P], identb[:])
                        eng.tensor_copy(
                            dst[:, 4 * g:4 * g + 4, rt * P:(rt + 1) * P],
                            tp.rearrange("p (o q) -> p o q", q=P))

            def col_side(t, chunk):
                """S21_hat columns [NTOK, R_c] -> fp8 allgather payload."""
                ag_in = dram2.tile([NTOK, RPC], FP8, tag="agin", name=f"agin{t}")
                ag_out = dram2.tile([NC * NTOK, RPC], FP8, tag="agout",
                                    addr_space="Shared", name=f"agout{t}")
                at_sb = atp.tile([P, NKB, RPC], BF16, tag="at", name=f"at{t}")
                transpose_quarters(t, chunk, at_sb, "c")
                # local col-max -> AllReduce max
                cm_loc = smp.tile([P, NKB], F32, tag="sm", name=f"cml{t}")
                nc.vector.reduce_max(cm_loc[:], at_sb[:], axis=AX.X)
                cm_in = dram3.tile([P, NKB], F32, tag="cmin", name=f"cmin{t}")
                cm_out = dram3.tile([P, NKB], F32, tag="cmout", addr_space="Shared",
                                    name=f"cmout{t}")
                nc.sync.dma_start(cm_in[:], cm_loc[:])
                nc.gpsimd.collective_compute(
                    "AllReduce", OP.max, replica_groups=[list(range(NC))],
                    ins=[cm_in[:]], outs=[cm_out[:]])
                return ag_in, ag_out, at_sb, cm_loc, cm_out

            def col_exp(t, at_sb, cm_loc):
                """exp in place against LOCAL col-max (collective-free), and
                local col-sums. Global correction happens in col_pack."""
                nbias = smp.tile([P, NKB], F32, tag="sm", name=f"nb{t}")
                nc.vector.tensor_scalar_mul(nbias[:], cm_loc[:], -SCALE)
                ls_loc = smp.tile([P, NKB], F32, tag="sm", name=f"lsl{t}")
                for kb in range(NKB):
                    nc.scalar.activation(
                        at_sb[:, kb, :], at_sb[:, kb, :], ACT.Exp,
                        bias=nbias[:, kb:kb + 1], scale=SCALE,
                        accum_out=ls_loc[:, kb:kb + 1])
                return ls_loc

            def col_correct(t, cm_loc, cm_out, ls_loc):
                """g = exp(S*(cm_loc - cm_glob)); AR-add of g*lsum."""
                cmg = smp.tile([P, NKB], F32, tag="sm", name=f"cmg{t}")
                nc.sync.dma_start(cmg[:], cm_out[:])
                gcor = smp.tile([P, NKB], F32, tag="sm", name=f"gc{t}")
                nc.vector.tensor_tensor(gcor[:], cm_loc[:], cmg[:], op=OP.subtract)
                nc.scalar.activation(gcor[:], gcor[:], ACT.Exp, bias=0.0,
                                     scale=SCALE)
                gls = smp.tile([P, NKB], F32, tag="sm", name=f"gls{t}")
                nc.vector.tensor_tensor(gls[:], gcor[:], ls_loc[:], op=OP.mult)
                cs_in = dram3.tile([P, NKB], F32, tag="csin", name=f"csin{t}")
                cs_out = dram3.tile([P, NKB], F32, tag="csout", addr_space="Shared",
                                    name=f"csout{t}")
                nc.sync.dma_start(cs_in[:], gls[:])
                nc.gpsimd.collective_compute(
                    "AllReduce", OP.add, replica_groups=[list(range(NC))],
                    ins=[cs_in[:]], outs=[cs_out[:]])
                return gcor, cs_out

            def col_pack(t, ag_in, ag_out, at_sb, gcor, cs_out):
                """payload = at_sb * (g/CS_glob) -> fp8 -> AllGather."""
                csg = smp.tile([P, NKB], F32, tag="sm", name=f"csg{t}")
                nc.sync.dma_start(csg[:], cs_out[:])
                csinv = smp.tile([P, NKB], F32, tag="sm", name=f"csi{t}")
                nc.vector.reciprocal(csinv[:], csg[:])
                fac = smp.tile([P, NKB], F32, tag="sm", name=f"fac{t}")
                nc.vector.tensor_tensor(fac[:], gcor[:], csinv[:], op=OP.mult)
                for h in range(4):
                    stg = stgp.tile([P, 8, RPC], FP8, tag="stg", name=f"stg{t}_{h}")
                    nc.gpsimd.tensor_tensor(
                        stg[:], at_sb[:, 8 * h:8 * h + 8, :],
                        fac[:, 8 * h:8 * h + 8, None]
                        .to_broadcast((P, 8, RPC)), op=OP.mult)
                    nc.sync.dma_start(
                        ag_in[h * 8 * P:(h + 1) * 8 * P, :]
                        .rearrange("(o p) n -> p o n", p=P), stg[:])
                nc.gpsimd.collective_compute(
                    "AllGather", OP.bypass, replica_groups=[list(range(NC))],
                    ins=[ag_in[:]], outs=[ag_out[:]])

            def row_side(t, chunk):
                """row softmax in place (bf16), then transpose -> pt fp8."""
                for rt in range(NRT):
                    rm = smp.tile([P, 1], F32, tag="sm", name=f"rm_{t}_{rt}")
                    nc.vector.reduce_max(rm[:], chunk[rt][:], axis=AX.X)
                    bias = smp.tile([P, 1], F32, tag="sm", name=f"bias_{t}_{rt}")
                    nc.vector.tensor_scalar_mul(bias[:], rm[:], -SCALE)
                    ssum = smp.tile([P, 1], F32, tag="sm", name=f"ss_{t}_{rt}")
                    nc.scalar.activation(chunk[rt][:], chunk[rt][:], ACT.Exp,
                                         bias=bias[:], scale=SCALE,
                                         accum_out=ssum[:])
                    rs = smp.tile([P, 1], F32, tag="sm", name=f"rs_{t}_{rt}")
                    nc.vector.reciprocal(rs[:], ssum[:])
                    nc.scalar.activation(chunk[rt][:], chunk[rt][:], ACT.Copy,
                                         bias=0.0, scale=rs[:])
                pt = ptp.tile([P, NKB, RPC], FP8, tag="pt", name=f"pt{t}")
                transpose_quarters(t, chunk, pt, "r")
                return pt

            def m_phase(u, pt, ag_out):
                """M^T tiles = (S12_hat @ S21_hat)^T[jtile, R_c]; stats to out[u].

                DoubleRow fp8: stationary [128, 2, 128] from the gathered
                S21 payload (k-major), moving pt [128, 2, 512].
                """
                racc = stp.tile([P, RPC], F32, tag="racc", name=f"racc{u}")
                nc.vector.memset(racc[:], 0.0)
                dvallw = stp.tile([P, NRT, 32], F32, tag="dvall", name=f"dvall{u}")
                cm32 = stp.tile([P, 32], F32, tag="cm32", name=f"cm32{u}")
                nwselJ = nwsel.rearrange("p (j b) -> p j b", b=NRT)
                for jg in range(NC):
                    pss = [psM.tile([P, 512], F32, tag="psM", name=f"mps{u}_{jg}_{j2}")
                           for j2 in range(4)]
                    for kb2 in range(NKB // 2):
                        qsb = qsbp.tile([P, 2, RPC], FP8, tag="qsb",
                                        name=f"qs{u}_{jg}_{kb2}")
                        nc.sync.dma_start(
                            qsb[:], ag_out[jg * NTOK + kb2 * 2 * P:
                                           jg * NTOK + (kb2 + 1) * 2 * P, :]
                            .rearrange("(o p) n -> p o n", p=P))
                        for j2 in range(4):
                            nc.tensor.matmul(
                                pss[j2][:], qsb[:, :, j2 * P:(j2 + 1) * P],
                                pt[:, 2 * kb2:2 * kb2 + 2, :],
                                start=(kb2 == 0), stop=(kb2 == NKB // 2 - 1),
                                perf_mode=DR)
                    for j2 in range(4):
                        j = 4 * jg + j2
                        msb = stp.tile([P, 512], F32, tag="msb", name=f"msb{u}_{j}")
                        nc.scalar.copy(msb[:], pss[j2][:])
                        tmp4 = stp.tile([P, 512], F32, tag="tmp4", name=f"t4_{u}_{j}")
                        nc.vector.tensor_tensor(tmp4[:], msb[:], imask4[:], op=OP.mult)
                        dv4 = smp.tile([P, NRT], F32, tag="sm4", name=f"dv4_{u}_{j}")
                        nc.vector.reduce_sum(
                            dv4[:], tmp4.rearrange("p (b q) -> p b q", q=P), axis=AX.X)
                        dv4w = smp.tile([P, NRT], F32, tag="sm4", name=f"dvw_{u}_{j}")
                        nc.vector.tensor_tensor(dv4w[:], dv4[:], nwselJ[:, j, :],
                                                op=OP.mult)
                        nc.vector.tensor_copy(dvallw[:, :, j], dv4w[:])
                        sc = stp.tile([P, NRT, P], F32, tag="tmp4", name=f"sc_{u}_{j}")
                        nc.vector.tensor_tensor(
                            sc[:], imask4[:],
                            dv4w[:, :, None].to_broadcast((P, NRT, P)), op=OP.mult)
                        nc.vector.tensor_add(
                            msb.rearrange("p (b q) -> p b q", q=P), msb.rearrange(
                                "p (b q) -> p b q", q=P), sc[:])
                        nc.vector.reduce_max(cm32[:, j:j + 1], msb[:], axis=AX.X)
                        nc.vector.tensor_tensor(racc[:], racc[:], msb[:], op=OP.max)
                # diag output = -sum_j dvallw
                dsum = smp.tile([P, NRT], F32, tag="sm4", name=f"dsum{u}")
                nc.vector.reduce_sum(dsum[:], dvallw[:], axis=AX.X)
                diag = smp.tile([P, NRT], F32, tag="sm4", name=f"diag{u}")
                nc.vector.tensor_scalar_mul(diag[:], dsum[:], -1.0)
                nc.sync.dma_start(out[u, :, 0:RPC], racc[:])
                nc.sync.dma_start(out[u, :, RPC:RPC + 32], cm32[:])
                nc.sync.dma_start(out[u, :, RPC + 32:OUT_W], diag[:])

            # ---------------- main pipeline (2-term m_phase lag) ----------------
            pending = []   # [(u, pt, ag_out), ...]
            for t, (is_tri, gk, la, ra) in enumerate(TERMS):
                if is_tri:
                    lhs_a = compute_ut(gk, la, f"a{t}")
                else:
                    lhs_a = load_xb(la, f"a{t}")
                chunk = side_chunk(t, lhs_a, ra)
                ag_in, ag_out, at_sb, cm_loc, cm_out = col_side(t, chunk)
                ls_loc = col_exp(t, at_sb, cm_loc)
                pt = row_side(t, chunk)
                if len(pending) >= 2:
                    m_phase(*pending.pop(0))
                gcor, cs_out = col_correct(t, cm_loc, cm_out, ls_loc)
                col_pack(t, ag_in, ag_out, at_sb, gcor, cs_out)
                if t in (1, 2, 3):
                    kick_gram_ar(3 - t)
                pending.append((t, pt, ag_out))
            for args in pending:
                m_phase(*args)

    nc.finalize()
    return nc


_PROGRAM = None


def _get_program():
    global _PROGRAM
    if _PROGRAM is None:
        _PROGRAM = build_program()
    return _PROGRAM


def _normalize(x):
    n = np.linalg.norm(x.astype(np.float32), axis=-1, keepdims=True)
    return (x / np.maximum(n, 1e-12)).astype(np.float32)


def _build_in_maps(inputs):
    nf = [_normalize(np.asarray(inputs[k], np.float32))
          for k in ("feat0", "feat1", "feat2")]
    nfT = [np.ascontiguousarray(x.T) for x in nf]

    import ml_dtypes
    nfTb = [x.astype(ml_dtypes.bfloat16) for x in nfT]
    in_maps = []
    for c in range(NC):
        rows = slice(c * RPC, (c + 1) * RPC)
        m = {}
        for i in range(3):
            m[f"x{i}"] = np.ascontiguousarray(nfT[i][:, rows])
            m[f"xb{i}"] = np.ascontiguousarray(nfTb[i][:, rows])
            m[f"w{i}"] = np.ascontiguousarray(nf[i][rows])
            m[f"f{i}"] = nfTb[i]
        wsel = np.zeros((P, P), np.float32)
        for b in range(NRT):
            j = 4 * c + b
            wsel[:, 4 * j + b] = 1.0     # wselJ[p, j, b] layout
        m["wsel"] = wsel
        in_maps.append(m)
    return in_maps


def _reduce(results):
    """results: list (per core) of {'out': [6, 128, OUT_W]} -> scalar loss."""
    L = np.zeros(6, np.float64)
    for t in range(6):
        rowpart = 0.0
        colmax = np.full(NTOK, -np.inf)
        diag_g = np.zeros(NTOK)
        for c in range(NC):
            o = results[c]["out"][t].astype(np.float64)
            racc = o[:, 0:RPC]
            cm32 = o[:, RPC:RPC + 32]
            dacc = o[:, RPC + 32:OUT_W]
            rowmax_local = racc.max(axis=0)                   # [512]
            diag_local = dacc.T.reshape(RPC)                  # [512]
            rowpart += np.maximum(rowmax_local + MARGIN - diag_local, 0.0).sum()
            colmax = np.maximum(colmax, cm32.T.reshape(NTOK))
            diag_g[c * RPC:(c + 1) * RPC] = diag_local
        colpart = np.maximum(colmax + MARGIN - diag_g, 0.0).sum()
        L[t] = (rowpart + colpart) / (2.0 * NTOK)
    loss = (L[0] + L[1] + L[2]) / 3.0 + (L[3] + L[4] + L[5]) / 3.0
    return np.float32(loss)


def kernel(feat0, feat1, feat2):
    in_maps = _build_in_maps({"feat0": feat0, "feat1": feat1, "feat2": feat2})
    nc = _get_program()
    res = run_bass_kernel_spmd(nc, in_maps, core_ids=list(range(NC)))
    return _reduce(res.results)


if __name__ == "__main__":
    rng = np.random.default_rng(0)
    f0 = rng.standard_normal((NTOK, D), dtype=np.float32)
    f1 = rng.standard_normal((NTOK, D), dtype=np.float32)
    f2 = rng.standard_normal((NTOK, D), dtype=np.float32)
    print("loss:", kernel(f0, f1, f2))


# revision 28
# speedup vs baseline: 1.1546x; 1.0028x over previous
"""Trainium2 Bass kernel for the pairwise+triplewise cycle-consistency loss.

Strategy (8 NeuronCores, tensor-parallel over rows of each [N,N] block):
  - All six cycle-term matrices have the form  A = U @ nf_j^T  with
    U = nf_i (pairs) or U = nf_i @ G_k (triples), G_k = nf_k^T nf_k [D,D],
    collapsing the [N,N]@[N,N] triple products into [D,D] Gram matmuls.
  - Each core owns a 512-row block R_c and computes A[R_c,:] ONCE (f32r
    matmuls, bf16 result). S12_hat rows come from a local row-softmax.
    S21_hat (the column softmax) is derived from PE-transposed A tiles
    plus two tiny [128,32] AllReduces (col-max, col-sum) — the second
    [RPC,D]@[D,N] matmul set of the baseline is gone.
  - Both M-product operands are quantized to fp8e4 (values in [0,1]);
    S21_hat columns are AllGathered as a [N, RPC] fp8 payload and the
    M^T column-tiles are computed with DoubleRow fp8 matmuls (2x rate),
    accumulating rowmax/colmax/diag stats on the fly. Host assembles the
    scalar loss.
"""
import sys
sys.path.insert(0, "/opt/trn_rl_repo")

import math
import numpy as np

import concourse.bass as bass
import concourse.mybir as mybir
import concourse.tile as tile
from concourse import bacc
from concourse.bass_utils import run_bass_kernel_spmd
from concourse.masks import make_identity

F32 = mybir.dt.float32
F32R = mybir.dt.float32r
BF16 = mybir.dt.bfloat16
FP8 = mybir.dt.float8e4
AX = mybir.AxisListType
OP = mybir.AluOpType
ACT = mybir.ActivationFunctionType
DR = mybir.MatmulPerfMode.DoubleRow

NTOK = 4096          # rows per view
D = 1024             # feature dim
NC = 8               # cores
RPC = NTOK // NC     # rows per core (512)
P = 128
NRT = RPC // P       # rowtiles per core (4)
NS = 8               # 512-col strips of A
DKB = D // P         # d-blocks (8)
NKB = NTOK // P      # k-tiles (32)
SCALE = math.log(NTOK) / 0.1
MARGIN = 0.5

# term table: (is_tri, gram_idx, lhsA, rhsA); lhs indexes x_i, rhs indexes f_i.
# For tri terms lhs is G[gram_idx] @ x_i.
TERMS = [
    (False, None, 0, 1),   # S01
    (False, None, 0, 2),   # S02
    (False, None, 1, 2),   # S12
    (True, 2, 0, 1),       # S02 @ S21 = nf0 G2 nf1^T
    (True, 1, 0, 2),       # S01 @ S12 = nf0 G1 nf2^T
    (True, 0, 1, 2),       # S10 @ S02 = nf1 G0 nf2^T
]

OUT_W = RPC + 32 + NRT   # racc 512 | colmax32 32 | diag 4


def build_program():
    nc = bacc.Bacc("TRN2", target_bir_lowering=False, debug=False, num_devices=NC)

    xs = [nc.dram_tensor(f"x{i}", [D, RPC], F32R, kind="ExternalInput") for i in range(3)]
    xbs = [nc.dram_tensor(f"xb{i}", [D, RPC], BF16, kind="ExternalInput") for i in range(3)]
    ws = [nc.dram_tensor(f"w{i}", [RPC, D], F32R, kind="ExternalInput") for i in range(3)]
    fs = [nc.dram_tensor(f"f{i}", [D, NTOK], BF16, kind="ExternalInput") for i in range(3)]
    wsel_in = nc.dram_tensor("wsel", [P, P], F32, kind="ExternalInput")
    out = nc.dram_tensor("out", [6, P, OUT_W], F32, kind="ExternalOutput")

    with tile.TileContext(nc) as tc:
        with (
            tc.tile_pool(name="cst", bufs=1) as cst,
            tc.tile_pool(name="lhs", bufs=2) as lhsp,
            tc.tile_pool(name="rhs", bufs=2) as rhsp,
            tc.tile_pool(name="abf", bufs=4) as abfp,
            tc.tile_pool(name="at", bufs=1) as atp,
            tc.tile_pool(name="pt", bufs=3) as ptp,
            tc.tile_pool(name="stg", bufs=2) as stgp,
            tc.tile_pool(name="qsb", bufs=3) as qsbp,
            tc.tile_pool(name="st", bufs=2) as stp,
            tc.tile_pool(name="sm", bufs=4) as smp,
            tc.tile_pool(name="psA", bufs=2, space="PSUM") as psA,
            tc.tile_pool(name="psT", bufs=2, space="PSUM") as psT,
            tc.tile_pool(name="psM", bufs=4, space="PSUM") as psM,
            tc.tile_pool(name="dram", bufs=1, space="DRAM") as dram,
            tc.tile_pool(name="dram2", bufs=3, space="DRAM") as dram2,
            tc.tile_pool(name="dram3", bufs=2, space="DRAM") as dram3,
        ):
            # constants
            identb = cst.tile([P, P], BF16)
            make_identity(nc, identb)
            wsel = cst.tile([P, P], F32)
            nc.sync.dma_start(wsel[:], wsel_in[:])
            nwsel = cst.tile([P, P], F32)
            nc.vector.tensor_scalar_mul(nwsel[:], wsel[:], -1.0)
            # imask4[p, 128b+p] = 1 for b in 0..3 (diag candidate positions)
            identf = cst.tile([P, P], F32)
            make_identity(nc, identf)
            imask4 = cst.tile([P, NRT, P], F32)
            for b in range(NRT):
                nc.vector.tensor_copy(imask4[:, b, :], identf[:])

            # ---------------- Gram phase ----------------
            gins = [dram.tile([D, D], F32, tag=f"gin{k}", name=f"gin{k}")
                    for k in range(3)]
            gouts = [dram.tile([D, D], F32, tag=f"gout{k}", addr_space="Shared",
                               name=f"gout{k}") for k in range(3)]
            for k in range(3):
                w_sb = lhsp.tile([P, NRT, D], F32R, tag="lhs", name=f"w_sb{k}")
                nc.sync.dma_start(w_sb[:], ws[k].rearrange("(o p) d -> p o d", p=P))
                for d1 in range(DKB):
                    for d2 in range(2):
                        ps = psA.tile([P, 512], F32, tag="psA", name=f"gps{k}_{d1}_{d2}")
                        for nt in range(NRT):
                            nc.tensor.matmul(
                                ps[:], w_sb[:, nt, d1 * P:(d1 + 1) * P],
                                w_sb[:, nt, d2 * 512:(d2 + 1) * 512],
                                start=(nt == 0), stop=(nt == NRT - 1))
                        gtmp = stp.tile([P, 512], F32, tag="msb", name=f"gt{k}_{d1}_{d2}")
                        nc.scalar.copy(gtmp[:], ps[:])
                        nc.sync.dma_start(
                            gins[k][d1 * P:(d1 + 1) * P,
                                    d2 * 512:(d2 + 1) * 512], gtmp[:])

            def kick_gram_ar(k):
                nc.gpsimd.collective_compute(
                    "AllReduce", OP.add, replica_groups=[list(range(NC))],
                    ins=[gins[k][:]], outs=[gouts[k][:]])

            # ---------------- helpers ----------------
            def load_xb(i, nm):
                t = lhsp.tile([P, DKB, RPC], BF16, tag="lhs", name=f"xb_{nm}")
                nc.sync.dma_start(t[:], xbs[i].rearrange("(o p) r -> p o r", p=P))
                return t

            def compute_ut(gk, i, nm):
                """U^T[:, R_c] = G_k @ x_i  -> [128, DKB, RPC] bf16 tile."""
                x_sb = lhsp.tile([P, DKB, RPC], F32R, tag="lhs", name=f"utx_{nm}")
                nc.sync.dma_start(x_sb[:], xs[i].rearrange("(o p) r -> p o r", p=P))
                ut = lhsp.tile([P, DKB, RPC], BF16, tag="lhs", name=f"ut_{nm}")
                for grp in range(2):
                    pss = [psM.tile([P, 512], F32, tag="psM", name=f"utps_{nm}_{grp}_{d4}")
                           for d4 in range(4)]
                    for half in range(2):
                        gh = rhsp.tile([P, 4, D], F32R, tag="rhs", name=f"gh_{nm}_{grp}_{half}")
                        nc.sync.dma_start(
                            gh[:], gouts[gk][half * 512:(half + 1) * 512]
                            .rearrange("(o p) d -> p o d", p=P).bitcast(F32R))
                        for d4 in range(4):
                            dp = 4 * grp + d4
                            for db in range(4):
                                nc.tensor.matmul(
                                    pss[d4][:], gh[:, db, dp * P:(dp + 1) * P],
                                    x_sb[:, 4 * half + db, :],
                                    start=(half == 0 and db == 0),
                                    stop=(half == 1 and db == 3))
                    for d4 in range(4):
                        nc.scalar.copy(ut[:, 4 * grp + d4, :], pss[d4][:])
                return ut

            def side_chunk(t, lhs_t, fj):
                """A[R_c, :] raw logits (pre-scale) as 4 bf16 quarter tiles."""
                chunk = [abfp.tile([P, NTOK], BF16, tag="abf", name=f"ch_{t}_{rt}")
                         for rt in range(NRT)]
                for s in range(NS):
                    rsb = rhsp.tile([P, DKB, 512], BF16, tag="rhs", name=f"rs_{t}_{s}")
                    nc.sync.dma_start(
                        rsb[:], fs[fj][:, s * 512:(s + 1) * 512]
                        .rearrange("(o p) n -> p o n", p=P))
                    for rt in range(NRT):
                        ps = psA.tile([P, 512], F32, tag="psA", name=f"aps_{t}_{s}_{rt}")
                        for kb in range(DKB):
                            nc.tensor.matmul(
                                ps[:], lhs_t[:, kb, rt * P:(rt + 1) * P],
                                rsb[:, kb, :], start=(kb == 0), stop=(kb == DKB - 1))
                        nc.vector.tensor_copy(chunk[rt][:, s * 512:(s + 1) * 512],
                                              ps[:])
                return chunk

            def transpose_quarters(t, chunk, dst, nm, eng=None):
                """PE-transpose chunk[rt] (4x [P, NTOK] bf16) into dst [P, NKB, RPC]."""
                eng = eng or nc.vector
                for rt in range(NRT):
                    for g in range(NKB // 4):
                        tp = psT.tile([P, 512], BF16, tag="psT", name=f"tp{nm}_{t}_{rt}_{g}")
                        for q in range(4):
                            kb = 4 * g + q
                            nc.tensor.transpose(
                                tp[:, q * P:(q + 1) * P],
                                chunk[rt][:, kb * P:(kb + 1) * P], identb[:])
                        eng.tensor_copy(
                            dst[:, 4 * g:4 * g + 4, rt * P:(rt + 1) * P],
                            tp.rearrange("p (o q) -> p o q", q=P))

            def col_side(t, chunk):
                """S21_hat columns [NTOK, R_c] -> fp8 allgather payload."""
                ag_in = dram2.tile([NTOK, RPC], FP8, tag="agin", name=f"agin{t}")
                ag_out = dram2.tile([NC * NTOK, RPC], FP8, tag="agout",
                                    addr_space="Shared", name=f"agout{t}")
                at_sb = atp.tile([P, NKB, RPC], BF16, tag="at", name=f"at{t}")
                transpose_quarters(t, chunk, at_sb, "c")
                # local col-max -> AllReduce max
                cm_loc = smp.tile([P, NKB], F32, tag="sm", name=f"cml{t}")
                nc.vector.reduce_max(cm_loc[:], at_sb[:], axis=AX.X)
                cm_in = dram3.tile([P, NKB], F32, tag="cmin", name=f"cmin{t}")
                cm_out = dram3.tile([P, NKB], F32, tag="cmout", addr_space="Shared",
                                    name=f"cmout{t}")
                nc.sync.dma_start(cm_in[:], cm_loc[:])
                nc.gpsimd.collective_compute(
                    "AllReduce", OP.max, replica_groups=[list(range(NC))],
                    ins=[cm_in[:]], outs=[cm_out[:]])
                return ag_in, ag_out, at_sb, cm_loc, cm_out

            def col_exp(t, at_sb, cm_loc):
                """exp in place against LOCAL col-max (collective-free), and
                local col-sums. Global correction happens in col_pack."""
                nbias = smp.tile([P, NKB], F32, tag="sm", name=f"nb{t}")
                nc.vector.tensor_scalar_mul(nbias[:], cm_loc[:], -SCALE)
                ls_loc = smp.tile([P, NKB], F32, tag="sm", name=f"lsl{t}")
                for kb in range(NKB):
                    nc.scalar.activation(
                        at_sb[:, kb, :], at_sb[:, kb, :], ACT.Exp,
                        bias=nbias[:, kb:kb + 1], scale=SCALE,
                        accum_out=ls_loc[:, kb:kb + 1])
                return ls_loc

            def col_correct(t, cm_loc, cm_out, ls_loc):
                """g = exp(S*(cm_loc - cm_glob)); AR-add of g*lsum."""
                cmg = smp.tile([P, NKB], F32, tag="sm", name=f"cmg{t}")
                nc.sync.dma_start(cmg[:], cm_out[:])
                gcor = smp.tile([P, NKB], F32, tag="sm", name=f"gc{t}")
                nc.vector.tensor_tensor(gcor[:], cm_loc[:], cmg[:], op=OP.subtract)
                nc.scalar.activation(gcor[:], gcor[:], ACT.Exp, bias=0.0,
                                     scale=SCALE)
                gls = smp.tile([P, NKB], F32, tag="sm", name=f"gls{t}")
                nc.vector.tensor_tensor(gls[:], gcor[:], ls_loc[:], op=OP.mult)
                cs_in = dram3.tile([P, NKB], F32, tag="csin", name=f"csin{t}")
                cs_out = dram3.tile([P, NKB], F32, tag="csout", addr_space="Shared",
                                    name=f"csout{t}")
                nc.sync.dma_start(cs_in[:], gls[:])
                nc.gpsimd.collective_compute(
                    "AllReduce", OP.add, replica_groups=[list(range(NC))],
                    ins=[cs_in[:]], outs=[cs_out[:]])
                return gcor, cs_out

            def col_pack(t, ag_in, ag_out, at_sb, gcor, cs_out):
                """payload = at_sb * (g/CS_glob) -> fp8 -> AllGather."""
                csg = smp.tile([P, NKB], F32, tag="sm", name=f"csg{t}")
                nc.sync.dma_start(csg[:], cs_out[:])
                csinv = smp.tile([P, NKB], F32, tag="sm", name=f"csi{t}")
                nc.vector.reciprocal(csinv[:], csg[:])
                fac = smp.tile([P, NKB], F32, tag="sm", name=f"fac{t}")
                nc.vector.tensor_tensor(fac[:], gcor[:], csinv[:], op=OP.mult)
                for h in range(4):
                    stg = stgp.tile([P, 8, RPC], FP8, tag="stg", name=f"stg{t}_{h}")
                    nc.gpsimd.tensor_tensor(
                        stg[:], at_sb[:, 8 * h:8 * h + 8, :],
                        fac[:, 8 * h:8 * h + 8, None]
                        .to_broadcast((P, 8, RPC)), op=OP.mult)
                    nc.sync.dma_start(
                        ag_in[h * 8 * P:(h + 1) * 8 * P, :]
                        .rearrange("(o p) n -> p o n", p=P), stg[:])
                nc.gpsimd.collective_compute(
                    "AllGather", OP.bypass, replica_groups=[list(range(NC))],
                    ins=[ag_in[:]], outs=[ag_out[:]])

            def row_side(t, chunk):
                """row softmax in place (bf16), then transpose -> pt fp8."""
                for rt in range(NRT):
                    rm = smp.tile([P, 1], F32, tag="sm", name=f"rm_{t}_{rt}")
                    nc.vector.reduce_max(rm[:], chunk[rt][:], axis=AX.X)
                    bias = smp.tile([P, 1], F32, tag="sm", name=f"bias_{t}_{rt}")
                    nc.vector.tensor_scalar_mul(bias[:], rm[:], -SCALE)
                    ssum = smp.tile([P, 1], F32, tag="sm", name=f"ss_{t}_{rt}")
                    nc.scalar.activation(chunk[rt][:], chunk[rt][:], ACT.Exp,
                                         bias=bias[:], scale=SCALE,
                                         accum_out=ssum[:])
                    rs = smp.tile([P, 1], F32, tag="sm", name=f"rs_{t}_{rt}")
                    nc.vector.reciprocal(rs[:], ssum[:])
                    nc.scalar.activation(chunk[rt][:], chunk[rt][:], ACT.Copy,
                                         bias=0.0, scale=rs[:])
                pt = ptp.tile([P, NKB, RPC], FP8, tag="pt", name=f"pt{t}")
                transpose_quarters(t, chunk, pt, "r")
                return pt

            def m_phase(u, pt, ag_out):
                """M^T tiles = (S12_hat @ S21_hat)^T[jtile, R_c]; stats to out[u].

                DoubleRow fp8: stationary [128, 2, 128] from the gathered
                S21 payload (k-major), moving pt [128, 2, 512].
                """
                racc = stp.tile([P, RPC], F32, tag="racc", name=f"racc{u}")
                nc.vector.memset(racc[:], 0.0)
                dvallw = stp.tile([P, NRT, 32], F32, tag="dvall", name=f"dvall{u}")
                cm32 = stp.tile([P, 32], F32, tag="cm32", name=f"cm32{u}")
                nwselJ = nwsel.rearrange("p (j b) -> p j b", b=NRT)
                for jg in range(NC):
                    pss = [psM.tile([P, 512], F32, tag="psM", name=f"mps{u}_{jg}_{j2}")
                           for j2 in range(4)]
                    for kb2 in range(NKB // 2):
                        qsb = qsbp.tile([P, 2, RPC], FP8, tag="qsb",
                                        name=f"qs{u}_{jg}_{kb2}")
                        nc.sync.dma_start(
                            qsb[:], ag_out[jg * NTOK + kb2 * 2 * P:
                                           jg * NTOK + (kb2 + 1) * 2 * P, :]
                            .rearrange("(o p) n -> p o n", p=P))
                        for j2 in range(4):
                            nc.tensor.matmul(
                                pss[j2][:], qsb[:, :, j2 * P:(j2 + 1) * P],
                                pt[:, 2 * kb2:2 * kb2 + 2, :],
                                start=(kb2 == 0), stop=(kb2 == NKB // 2 - 1),
                                perf_mode=DR)
                    for j2 in range(4):
                        j = 4 * jg + j2
                        msb = stp.tile([P, 512], F32, tag="msb", name=f"msb{u}_{j}")
                        nc.scalar.copy(msb[:], pss[j2][:])
                        tmp4 = stp.tile([P, 512], F32, tag="tmp4", name=f"t4_{u}_{j}")
                        nc.vector.tensor_tensor(tmp4[:], msb[:], imask4[:], op=OP.mult)
                        dv4 = smp.tile([P, NRT], F32, tag="sm4", name=f"dv4_{u}_{j}")
                        nc.vector.reduce_sum(
                            dv4[:], tmp4.rearrange("p (b q) -> p b q", q=P), axis=AX.X)
                        dv4w = smp.tile([P, NRT], F32, tag="sm4", name=f"dvw_{u}_{j}")
                        nc.vector.tensor_tensor(dv4w[:], dv4[:], nwselJ[:, j, :],
                                                op=OP.mult)
                        nc.vector.tensor_copy(dvallw[:, :, j], dv4w[:])
                        sc = stp.tile([P, NRT, P], F32, tag="tmp4", name=f"sc_{u}_{j}")
                        nc.vector.tensor_tensor(
                            sc[:], imask4[:],
                            dv4w[:, :, None].to_broadcast((P, NRT, P)), op=OP.mult)
                        nc.vector.tensor_add(
                            msb.rearrange("p (b q) -> p b q", q=P), msb.rearrange(
                                "p (b q) -> p b q", q=P), sc[:])
                        nc.vector.reduce_max(cm32[:, j:j + 1], msb[:], axis=AX.X)
                        nc.vector.tensor_tensor(racc[:], racc[:], msb[:], op=OP.max)
                # diag output = -sum_j dvallw
                dsum = smp.tile([P, NRT], F32, tag="sm4", name=f"dsum{u}")
                nc.vector.reduce_sum(dsum[:], dvallw[:], axis=AX.X)
                diag = smp.tile([P, NRT], F32, tag="sm4", name=f"diag{u}")
                nc.vector.tensor_scalar_mul(diag[:], dsum[:], -1.0)
                nc.sync.dma_start(out[u, :, 0:RPC], racc[:])
                nc.sync.dma_start(out[u, :, RPC:RPC + 32], cm32[:])
                nc.sync.dma_start(out[u, :, RPC + 32:OUT_W], diag[:])

            # ---------------- main pipeline (2-term m_phase lag) ----------------
            pending = []   # [(u, pt, ag_out), ...]
            for t, (is_tri, gk, la, ra) in enumerate(TERMS):
                if is_tri:
                    lhs_a = compute_ut(gk, la, f"a{t}")
                else:
                    lhs_a = load_xb(la, f"a{t}")
                chunk = side_chunk(t, lhs_a, ra)
                ag_in, ag_out, at_sb, cm_loc, cm_out = col_side(t, chunk)
                ls_loc = col_exp(t, at_sb, cm_loc)
                pt = row_side(t, chunk)
                if len(pending) >= 2:
                    m_phase(*pending.pop(0))
                gcor, cs_out = col_correct(t, cm_loc, cm_out, ls_loc)
                col_pack(t, ag_in, ag_out, at_sb, gcor, cs_out)
                if t in (1, 2, 3):
                    kick_gram_ar(3 - t)
                pending.append((t, pt, ag_out))
            for args in pending:
                m_phase(*args)

    nc.finalize()
    return nc


_PROGRAM = None


def _get_program():
    global _PROGRAM
    if _PROGRAM is None:
        _PROGRAM = build_program()
    return _PROGRAM


def _normalize(x):
    n = np.linalg.norm(x.astype(np.float32), axis=-1, keepdims=True)
    return (x / np.maximum(n, 1e-12)).astype(np.float32)


def _build_in_maps(inputs):
    nf = [_normalize(np.asarray(inputs[k], np.float32))
          for k in ("feat0", "feat1", "feat2")]
    nfT = [np.ascontiguousarray(x.T) for x in nf]

    import ml_dtypes
    nfTb = [x.astype(ml_dtypes.bfloat16) for x in nfT]
    in_maps = []
    for c in range(NC):
        rows = slice(c * RPC, (c + 1) * RPC)
        m = {}
        for i in range(3):
            m[f"x{i}"] = np.ascontiguousarray(nfT[i][:, rows])
            m[f"xb{i}"] = np.ascontiguousarray(nfTb[i][:, rows])
            m[f"w{i}"] = np.ascontiguousarray(nf[i][rows])
            m[f"f{i}"] = nfTb[i]
        wsel = np.zeros((P, P), np.float32)
        for b in range(NRT):
            j = 4 * c + b
            wsel[:, 4 * j + b] = 1.0     # wselJ[p, j, b] layout
        m["wsel"] = wsel
        in_maps.append(m)
    return in_maps


def _reduce(results):
    """results: list (per core) of {'out': [6, 128, OUT_W]} -> scalar loss."""
    L = np.zeros(6, np.float64)
    for t in range(6):
        rowpart = 0.0
        colmax = np.full(NTOK, -np.inf)
        diag_g = np.zeros(NTOK)
        for c in range(NC):
            o = results[c]["out"][t].astype(np.float64)
            racc = o[:, 0:RPC]
            cm32 = o[:, RPC:RPC + 32]
            dacc = o[:, RPC + 32:OUT_W]
            rowmax_local = racc.max(axis=0)                   # [512]
            diag_local = dacc.T.reshape(RPC)                  # [512]
            rowpart += np.maximum(rowmax_local + MARGIN - diag_local, 0.0).sum()
            colmax = np.maximum(colmax, cm32.T.reshape(NTOK))
            diag_g[c * RPC:(c + 1) * RPC] = diag_local
        colpart = np.maximum(colmax + MARGIN - diag_g, 0.0).sum()
        L[t] = (rowpart + colpart) / (2.0 * NTOK)
    loss = (L[0] + L[1] + L[2]) / 3.0 + (L[3] + L[4] + L[5]) / 3.0
    return np.float32(loss)


def kernel(feat0, feat1, feat2):
    in_maps = _build_in_maps({"feat0": feat0, "feat1": feat1, "feat2": feat2})
    nc = _get_program()
    res = run_bass_kernel_spmd(nc, in_maps, core_ids=list(range(NC)))
    return _reduce(res.results)


if __name__ == "__main__":
    rng = np.random.default_rng(0)
    f0 = rng.standard_normal((NTOK, D), dtype=np.float32)
    f1 = rng.standard_normal((NTOK, D), dtype=np.float32)
    f2 = rng.standard_normal((NTOK, D), dtype=np.float32)
    print("loss:", kernel(f0, f1, f2))
